# revision 7
# baseline (speedup 1.0000x reference)
"""VQ codebook cross-entropy kernel for Trainium2 (8 NeuronCores, SPMD).

Math per batch row b (reference semantics):
  enc = (x_flat - mean)/max(std,1e-6) @ pca            [B, 256]
  logits = -(||enc||^2 + ||c_k||^2 - 2 enc.c_k)        [B, 4096]
  t_b = argmax_k logits_target
  loss = -mean(log_softmax(logits_pred)[b, t_b]); acc = mean(argmax logits_pred == t_b)

log_softmax and argmax are invariant to the per-row shift ||enc||^2 (the
max(dist2,0) clamp never fires: min dist2 > 500 for this data), so the
device works with u_neg = -(logits + ||enc||^2) = (x @ W2) @ centersT + dneg
where W2 = -2*pca/std and dneg = c2 - 2*b0@cT are folded on the host.

Device pipeline per core (2048 rows, data-parallel over batch):
  - x is split hi/lo into bf16 on the host; DMA xbar-transpose puts the
    feature dim on partitions.
  - PE: all-bf16 hi/lo compensated matmuls (float32r is broken in this
    toolchain: it corrupts the weight path of neighboring matmuls). Encode
    and cross both use 3-term hi/lo products (err ~1e-4, better than fp32);
    dneg is pre-accumulated into PSUM by a K=3 ones-matmul against a bf16
    h/m/l decomposition.
  - PSUM then holds u_neg directly: DVE reduce_min per 1024-wide quarter
    (online softmax), ACT exp with per-quarter shift + fused sum -> Z_q,
    DVE scalar_tensor_tensor (is_le mask * u_neg_pred) -> v_q extracts the
    pred logit at the target argmin.
  - Host combines the tiny [128, 192] per-core outputs into (loss, acc).
"""
import sys
import os

sys.path.insert(0, "/opt/trn_rl_repo")

import numpy as np
import ml_dtypes

BF = ml_dtypes.bfloat16
B, T, D = 16384, 64, 16
F = T * D            # 1024
P = 256              # pca dim
K = 4096             # prototypes
N_CORES = 8
BS = B // N_CORES    # 2048 rows per core
NT = BS // 128       # 16 b-subtiles of 128 rows
NCH = 4              # chunks of 512 rows
F_T = F // 128       # 8 f-blocks
NQ = 4               # 1024-wide K quarters (online softmax)

_CACHE = {}


def _build():
    import concourse.bacc as bacc
    import concourse.tile as tile
    from concourse import mybir

    f32 = mybir.dt.float32
    bf16 = mybir.dt.bfloat16
    Alu = mybir.AluOpType
    Act = mybir.ActivationFunctionType
    AX = mybir.AxisListType.X

    nc = bacc.Bacc("TRN2", target_bir_lowering=False, debug=False,
                   num_devices=N_CORES)

    xth_d = nc.dram_tensor("xth", [F, BS], bf16, kind="ExternalInput")
    xtl_d = nc.dram_tensor("xtl", [F, BS], bf16, kind="ExternalInput")
    xph_d = nc.dram_tensor("xph", [F, BS], bf16, kind="ExternalInput")
    xpl_d = nc.dram_tensor("xpl", [F, BS], bf16, kind="ExternalInput")
    w2h_d = nc.dram_tensor("w2h", [F, P], bf16, kind="ExternalInput")
    w2l_d = nc.dram_tensor("w2l", [F, P], bf16, kind="ExternalInput")
    cth_d = nc.dram_tensor("cth", [P, K], bf16, kind="ExternalInput")
    ctl_d = nc.dram_tensor("ctl", [P, K], bf16, kind="ExternalInput")
    dn_d = nc.dram_tensor("dneg3", [3, K], bf16, kind="ExternalInput")
    on_d = nc.dram_tensor("ones3", [3, 128], bf16, kind="ExternalInput")
    out_d = nc.dram_tensor("res", [128, 3 * NT * NQ], f32,
                           kind="ExternalOutput")

    with tile.TileContext(nc) as tc:
        with (
            tc.tile_pool(name="const", bufs=1) as constp,
            tc.tile_pool(name="xts", bufs=6) as xts,
            tc.tile_pool(name="encp", bufs=1, space="PSUM") as encp,
            tc.tile_pool(name="encs", bufs=16) as encs,
            tc.tile_pool(name="cpsum", bufs=3, space="PSUM") as cpsum,
            tc.tile_pool(name="ubuf", bufs=1) as ubuf,
            tc.tile_pool(name="dump", bufs=2) as dumpp,
            tc.tile_pool(name="msc", bufs=4) as msc,
            tc.tile_pool(name="resp", bufs=1) as resp,
        ):
            w2sb = {}
            for nm, dd in (("h", w2h_d), ("l", w2l_d)):
                t = constp.tile([128, F_T * P], bf16, tag=f"w2{nm}")
                for j in range(F_T):
                    nc.sync.dma_start(t[:, j * P:(j + 1) * P],
                                      dd.ap()[j * 128:(j + 1) * 128, :])
                w2sb[nm] = t
            ctsb = {}
            for nm, dd in (("h", cth_d), ("l", ctl_d)):
                for t_ in range(2):
                    c = constp.tile([128, K], bf16, tag=f"ct{nm}{t_}")
                    nc.sync.dma_start(c[:],
                                      dd.ap()[t_ * 128:(t_ + 1) * 128, :])
                    ctsb[(nm, t_)] = c
            dnsb = constp.tile([3, K], bf16, tag="dneg3")
            nc.sync.dma_start(dnsb[:], dn_d.ap())
            onsb = constp.tile([3, 128], bf16, tag="ones3")
            nc.sync.dma_start(onsb[:], on_d.ap())

            vq_all = resp.tile([128, NT * NQ], f32, tag="v")
            mq_all = resp.tile([128, NT * NQ], f32, tag="mq")
            zq_all = resp.tile([128, NT * NQ], f32, tag="z")

            # ---- encode: xT hi/lo via DMA transpose, 3-term bf16 matmul ----
            enc_tiles = {}
            xd = {("t", "h"): xth_d, ("t", "l"): xtl_d,
                  ("p", "h"): xph_d, ("p", "l"): xpl_d}
            for ch in range(NCH):
                r0 = ch * 512
                for name in ("t", "p"):
                    ep = encp.tile([128, 1024], f32)
                    terms = [("h", "h"), ("l", "h"), ("h", "l")]
                    for j in range(F_T):
                        xtile = {}
                        for part in ("h", "l"):
                            xx = xts.tile([128, 512], bf16)
                            nc.sync.dma_start(
                                xx[:],
                                xd[(name, part)].ap()[j * 128:(j + 1) * 128,
                                                      r0:r0 + 512])
                            xtile[part] = xx
                        for ti, (wp, xp_) in enumerate(terms):
                            for h in range(2):
                                nc.tensor.matmul(
                                    ep[:, h * 512:(h + 1) * 512],
                                    w2sb[wp][:, j * P + h * 128:
                                             j * P + (h + 1) * 128],
                                    xtile[xp_][:],
                                    start=(j == 0 and ti == 0),
                                    stop=(j == F_T - 1 and ti == len(terms) - 1))
                    for h in range(2):
                        eh = encs.tile([128, 512], bf16, tag="ench")
                        nc.scalar.copy(eh[:], ep[:, h * 512:(h + 1) * 512])
                        el = encs.tile([128, 512], bf16, tag="encl")
                        nc.vector.scalar_tensor_tensor(
                            out=el[:], in0=ep[:, h * 512:(h + 1) * 512],
                            scalar=0.0, in1=eh[:],
                            op0=Alu.bypass, op1=Alu.subtract)
                        enc_tiles[(name, ch, h, "h")] = eh
                        enc_tiles[(name, ch, h, "l")] = el

            # ---- cross + epilogue per 128-row subtile ----
            for it in range(NT):
                ch, sub = divmod(it, 4)
                u_t = ubuf.tile([128, K], f32, tag="ut")
                mt4 = msc.tile([128, NQ], f32, tag="mt4")
                mtf = msc.tile([128, 1], f32, tag="mtf")
                for name in ("t", "p"):
                    for q in range(NQ):
                        cp = cpsum.tile([128, 1024], f32)
                        for n2 in range(2):
                            kk = q * 1024 + n2 * 512
                            sl = slice(n2 * 512, (n2 + 1) * 512)
                            nc.tensor.matmul(cp[:, sl], onsb[:],
                                             dnsb[:, kk:kk + 512],
                                             start=True, stop=False)
                            cterms = [("h", "h"), ("l", "h"), ("h", "l")]
                            for kt in range(2):
                                for ci, (ep_, cp_) in enumerate(cterms):
                                    nc.tensor.matmul(
                                        cp[:, sl],
                                        enc_tiles[(name, ch, kt, ep_)][
                                            :, sub * 128:(sub + 1) * 128],
                                        ctsb[(cp_, kt)][:, kk:kk + 512],
                                        start=False,
                                        stop=(kt == 1 and ci == 2))
                        if name == "t":
                            nc.vector.tensor_reduce(mt4[:, q:q + 1], cp[:],
                                                    axis=AX, op=Alu.min)
                            nc.scalar.copy(u_t[:, q * 1024:(q + 1) * 1024],
                                           cp[:])
                            if q == NQ - 1:
                                nc.vector.tensor_reduce(mtf[:], mt4[:],
                                                        axis=AX, op=Alu.min)
                        else:
                            col = it * NQ + q
                            nc.vector.tensor_reduce(mq_all[:, col:col + 1],
                                                    cp[:], axis=AX, op=Alu.min)
                            ex = dumpp.tile([128, 1024], f32, tag="ex")
                            nc.scalar.activation(
                                ex[:], cp[:], Act.Exp,
                                bias=mq_all[:, col:col + 1], scale=-1.0,
                                accum_out=zq_all[:, col:col + 1])
                            dm = dumpp.tile([128, 1024], f32, tag="dm")
                            nc.vector.scalar_tensor_tensor(
                                out=dm[:],
                                in0=u_t[:, q * 1024:(q + 1) * 1024],
                                scalar=mtf[:],
                                in1=cp[:],
                                op0=Alu.is_le,
                                op1=Alu.mult,
                                accum_out=vq_all[:, col:col + 1])

            NTQ = NT * NQ
            nc.sync.dma_start(out_d.ap()[:, 0:NTQ], vq_all[:])
            nc.sync.dma_start(out_d.ap()[:, NTQ:2 * NTQ], mq_all[:])
            nc.sync.dma_start(out_d.ap()[:, 2 * NTQ:3 * NTQ], zq_all[:])

    nc.compile()
    return nc


def _prep_host(pred_actions, target_actions, centers, mean, std,
               pca_components):
    f32 = np.float32
    mean = np.asarray(mean, f32)
    std = np.asarray(std, f32)
    pca = np.asarray(pca_components, f32)
    centers = np.asarray(centers, f32)
    inv_std = (1.0 / np.maximum(std, 1e-6)).astype(f32)
    w2 = (pca * (-2.0 * inv_std)[:, None]).astype(f32)
    w2h = w2.astype(BF)
    w2l = (w2 - w2h.astype(f32)).astype(BF)
    b0 = (-(mean * inv_std)) @ pca                      # [P]
    c2 = np.einsum("kp,kp->k", centers, centers)
    dneg = (c2 - 2.0 * (b0 @ centers.T)).astype(f32)    # [K]
    dh = dneg.astype(BF)
    dm = (dneg - dh.astype(f32)).astype(BF)
    dl = (dneg - dh.astype(f32) - dm.astype(f32)).astype(BF)
    dneg3 = np.ascontiguousarray(np.stack([dh, dm, dl], axis=0))  # [3, K]
    ones3 = np.ones((3, 128), dtype=BF)
    ctf = np.ascontiguousarray(centers.T).astype(f32)   # [P, K]
    cth = ctf.astype(BF)
    ctl = (ctf - cth.astype(f32)).astype(BF)

    def split(x):
        # hi/lo bf16 split, pre-transposed to [N_CORES, F, BS]
        x = np.asarray(x, f32).reshape(B, F)
        h = x.astype(BF)
        l = (x - h.astype(f32)).astype(BF)

        def shard_t(a):
            return np.ascontiguousarray(
                a.reshape(N_CORES, BS, F).transpose(0, 2, 1))

        return shard_t(h), shard_t(l)

    xth, xtl = split(target_actions)
    xph, xpl = split(pred_actions)
    return xth, xtl, xph, xpl, w2h, w2l, cth, ctl, dneg3, ones3


def run_device(xth, xtl, xph, xpl, w2h, w2l, cth, ctl, dneg3, ones3):
    from concourse.bass_utils import run_bass_kernel_spmd
    if "nc" not in _CACHE:
        _CACHE["nc"] = _build()
    nc = _CACHE["nc"]
    in_maps = []
    for c in range(N_CORES):
        in_maps.append({
            "xth": xth[c], "xtl": xtl[c], "xph": xph[c], "xpl": xpl[c],
            "w2h": w2h, "w2l": w2l, "cth": cth, "ctl": ctl,
            "dneg3": dneg3, "ones3": ones3,
        })
    res = run_bass_kernel_spmd(nc, in_maps, list(range(N_CORES)))
    return [r["res"] for r in res.results]


def reduce_host(outs):
    NTQ = NT * NQ
    loss_sum = 0.0
    acc_sum = 0
    for o in outs:
        v = o[:, 0:NTQ].reshape(128, NT, NQ)
        mq = o[:, NTQ:2 * NTQ].reshape(128, NT, NQ)
        zq = o[:, 2 * NTQ:3 * NTQ].reshape(128, NT, NQ)
        vsum = v.sum(axis=2)                       # u_neg_p at target argmin
        mp = mq.min(axis=2)                        # final m_neg_p
        # log Z with global shift: sum_q Zq * exp(m_neg_p - m_neg_q)
        z = (zq.astype(np.float64) * np.exp(
            (mp[:, :, None] - mq).astype(np.float64))).sum(axis=2)
        loss_rows = np.log(z) + (vsum - mp).astype(np.float64)
        loss_sum += loss_rows.sum()
        acc_sum += int((vsum == mp).sum())
    loss = np.float32(loss_sum / B)
    acc = np.float32(acc_sum / B)
    return loss, acc


def kernel(pred_actions, target_actions, centers, mean, std, pca_components):
    prepped = _prep_host(pred_actions, target_actions, centers, mean, std,
                         pca_components)
    outs = run_device(*prepped)
    return reduce_host(outs)


# revision 11
# speedup vs baseline: 1.9662x; 1.9662x over previous
"""VQ codebook cross-entropy kernel for Trainium2 (8 NeuronCores, SPMD).

Math per batch row b (reference semantics):
  enc = (x_flat - mean)/max(std,1e-6) @ pca            [B, 256]
  logits = -(||enc||^2 + ||c_k||^2 - 2 enc.c_k)        [B, 4096]
  t_b = argmax_k logits_target
  loss = -mean(log_softmax(logits_pred)[b, t_b]); acc = mean(argmax logits_pred == t_b)

log_softmax and argmax are invariant to per-row shifts (the max(dist2,0)
clamp never fires: min dist2 > 500 for this data), so the device works with
u_neg = (x @ W2) @ centersT + dneg, where W2 = -2*pca/std and
dneg = c2 - 2*b0@cT - const are folded on the host (global constant shifts
are invariant too, so dneg is mean-centered for fp16 friendliness).

Device pipeline per core (2048 rows, data-parallel over batch):
  - x is split hi/lo into fp16 on the host and pre-transposed (feature dim
    on partitions). All matmuls are 16-bit: fp16 mantissa compensation
    gives u error sigma ~2e-3 .. 1e-4 depending on term count.
    (float32r is broken in this toolchain: it corrupts the weight path of
    neighboring matmuls. The DMA xbar transpose races its completion
    signal, hence host-side pre-transpose.)
  - dneg is pre-accumulated into PSUM by a K=3 ones-matmul against a bf16
    h/m/l decomposition (exact to ~3e-5).
  - PSUM then holds u_neg: DVE reduce_min per 2048-wide half (online
    softmax), ACT exp with per-half shift + fused sum -> Z_q, DVE
    scalar_tensor_tensor (is_le mask * u_neg_pred) -> v_q extracts the
    pred logit at the target argmin.
  - Host combines the tiny per-core outputs into (loss, acc).
"""
import sys
import os

sys.path.insert(0, "/opt/trn_rl_repo")

import numpy as np
import ml_dtypes

BF = ml_dtypes.bfloat16
F16 = np.float16
B, T, D = 16384, 64, 16
F = T * D            # 1024
P = 256              # pca dim
K = 4096             # prototypes
N_CORES = 8
BS = B // N_CORES    # 2048 rows per core
NT = BS // 128       # 16 b-subtiles of 128 rows
NCH = 4              # chunks of 512 rows
F_T = F // 128       # 8 f-blocks
NQ = 4               # 1024-wide K quarters (online softmax)
QW = K // NQ         # 2048

ENC_TERMS = 2        # fp16 terms for encode: 2 -> (h,h),(l,h); 3 adds (h,l)
CROSS_TERMS = 1      # fp16 terms for cross: 1 -> (h,h); 2 adds (h,l); 3 adds (l,h)

_CACHE = {}


def _build():
    import concourse.bacc as bacc
    import concourse.tile as tile
    from concourse import mybir

    f32 = mybir.dt.float32
    bf16 = mybir.dt.bfloat16
    fp16 = mybir.dt.float16
    Alu = mybir.AluOpType
    Act = mybir.ActivationFunctionType
    AX = mybir.AxisListType.X

    nc = bacc.Bacc("TRN2", target_bir_lowering=False, debug=False,
                   num_devices=N_CORES)

    xth_d = nc.dram_tensor("xth", [F, BS], fp16, kind="ExternalInput")
    xtl_d = nc.dram_tensor("xtl", [F, BS], fp16, kind="ExternalInput")
    xph_d = nc.dram_tensor("xph", [F, BS], fp16, kind="ExternalInput")
    xpl_d = nc.dram_tensor("xpl", [F, BS], fp16, kind="ExternalInput")
    w2h_d = nc.dram_tensor("w2h", [F, P], fp16, kind="ExternalInput")
    w2l_d = nc.dram_tensor("w2l", [F, P], fp16, kind="ExternalInput")
    cth_d = nc.dram_tensor("cth", [P, K], fp16, kind="ExternalInput")
    ctl_d = nc.dram_tensor("ctl", [P, K], fp16, kind="ExternalInput")
    dn_d = nc.dram_tensor("dneg3", [3, K], bf16, kind="ExternalInput")
    on_d = nc.dram_tensor("ones3", [3, 128], bf16, kind="ExternalInput")
    out_d = nc.dram_tensor("res", [128, 3 * NT * NQ], f32,
                           kind="ExternalOutput")

    with tile.TileContext(nc) as tc:
        with (
            tc.tile_pool(name="const", bufs=1) as constp,
            tc.tile_pool(name="xts", bufs=10) as xts,
            tc.tile_pool(name="encs", bufs=16) as encs,
            tc.tile_pool(name="cpsum", bufs=3, space="PSUM") as cpsum,
            tc.tile_pool(name="encp", bufs=1, space="PSUM") as encp,
            tc.tile_pool(name="ubuf", bufs=3) as ubuf,
            tc.tile_pool(name="dump", bufs=4) as dumpp,
            tc.tile_pool(name="msc", bufs=10) as msc,
            tc.tile_pool(name="resp", bufs=1) as resp,
        ):
            w2sb = {}
            for nm, dd in (("h", w2h_d), ("l", w2l_d)):
                t = constp.tile([128, F_T * P], fp16, tag=f"w2{nm}")
                for j in range(F_T):
                    nc.sync.dma_start(t[:, j * P:(j + 1) * P],
                                      dd.ap()[j * 128:(j + 1) * 128, :])
                w2sb[nm] = t
            ctsb = {}
            ct_parts = ["h"] + (["l"] if CROSS_TERMS >= 2 else [])
            for nm, dd in (("h", cth_d), ("l", ctl_d)):
                if nm not in ct_parts:
                    continue
                for t_ in range(2):
                    c = constp.tile([128, K], fp16, tag=f"ct{nm}{t_}")
                    nc.sync.dma_start(c[:],
                                      dd.ap()[t_ * 128:(t_ + 1) * 128, :])
                    ctsb[(nm, t_)] = c
            dnsb = constp.tile([3, K], bf16, tag="dneg3")
            nc.sync.dma_start(dnsb[:], dn_d.ap())
            onsb = constp.tile([3, 128], bf16, tag="ones3")
            nc.sync.dma_start(onsb[:], on_d.ap())

            vq_all = resp.tile([128, NT * NQ], f32, tag="v")
            mq_all = resp.tile([128, NT * NQ], f32, tag="mq")
            zq_all = resp.tile([128, NT * NQ], f32, tag="z")

            # ---- encode: 2-3 term fp16 matmuls (pre-transposed x) ----
            enc_tiles = {}
            xd = {("t", "h"): xth_d, ("t", "l"): xtl_d,
                  ("p", "h"): xph_d, ("p", "l"): xpl_d}
            eterms = [("h", "h"), ("l", "h"), ("h", "l")][:ENC_TERMS]
            xparts = sorted({xp_ for (_, xp_) in eterms})
            def emit_encode(ch):
                r0 = ch * 512
                for name in ("t", "p"):
                    ep = encp.tile([128, 1024], f32, tag="ep")
                    for j in range(F_T):
                        xtile = {}
                        for part in xparts:
                            xx = xts.tile([128, 512], fp16)
                            nc.sync.dma_start(
                                xx[:],
                                xd[(name, part)].ap()[j * 128:(j + 1) * 128,
                                                      r0:r0 + 512])
                            xtile[part] = xx
                        for ti, (wp, xp_) in enumerate(eterms):
                            for h in range(2):
                                nc.tensor.matmul(
                                    ep[:, h * 512:(h + 1) * 512],
                                    w2sb[wp][:, j * P + h * 128:
                                             j * P + (h + 1) * 128],
                                    xtile[xp_][:],
                                    start=(j == 0 and ti == 0),
                                    stop=(j == F_T - 1 and ti == len(eterms) - 1))
                    for h in range(2):
                        eh = encs.tile([128, 512], fp16, tag="ench")
                        nc.scalar.copy(eh[:], ep[:, h * 512:(h + 1) * 512])
                        enc_tiles[(name, ch, h, "h")] = eh
                        if CROSS_TERMS >= 3:
                            el = encs.tile([128, 512], fp16, tag="encl")
                            nc.vector.scalar_tensor_tensor(
                                out=el[:], in0=ep[:, h * 512:(h + 1) * 512],
                                scalar=0.0, in1=eh[:],
                                op0=Alu.bypass, op1=Alu.subtract)
                            enc_tiles[(name, ch, h, "l")] = el

            cterms = [("h", "h"), ("h", "l"), ("l", "h")][:CROSS_TERMS]

            # ---- cross + epilogue per 128-row subtile, encode interleaved ----
            emit_encode(0)
            for it in range(NT):
                ch, sub = divmod(it, 4)
                if sub == 0 and ch + 1 < NCH:
                    emit_encode(ch + 1)
                u_t = ubuf.tile([128, K], f32, tag="ut")
                mt2 = msc.tile([128, NQ], f32, tag="mt2")
                mtf = msc.tile([128, 1], f32, tag="mtf")
                mqs = []
                for name in ("t", "p"):
                    for q in range(NQ):
                        cp = cpsum.tile([128, QW], f32, tag="cp")
                        for n2 in range(QW // 512):
                            kk = q * QW + n2 * 512
                            sl = slice(n2 * 512, (n2 + 1) * 512)
                            nc.tensor.matmul(cp[:, sl], onsb[:],
                                             dnsb[:, kk:kk + 512],
                                             start=True, stop=False)
                            for ci, (ep_, cp_) in enumerate(cterms):
                                for kt in range(2):
                                    nc.tensor.matmul(
                                        cp[:, sl],
                                        enc_tiles[(name, ch, kt, ep_)][
                                            :, sub * 128:(sub + 1) * 128],
                                        ctsb[(cp_, kt)][:, kk:kk + 512],
                                        start=False,
                                        stop=(ci == len(cterms) - 1 and kt == 1))
                        if name == "t":
                            nc.vector.tensor_reduce(mt2[:, q:q + 1], cp[:],
                                                    axis=AX, op=Alu.min)
                            nc.scalar.copy(u_t[:, q * QW:(q + 1) * QW], cp[:])
                            if q == NQ - 1:
                                nc.vector.tensor_reduce(mtf[:], mt2[:],
                                                        axis=AX, op=Alu.min)
                        else:
                            col = it * NQ + q
                            mq = msc.tile([128, 1], f32, tag="mq")
                            nc.vector.tensor_reduce(mq[:], cp[:],
                                                    axis=AX, op=Alu.min)
                            mqs.append((col, mq))
                            ex = dumpp.tile([128, QW], f32, tag="ex")
                            nc.scalar.activation(
                                ex[:], cp[:], Act.Exp,
                                bias=mq[:], scale=-1.0,
                                accum_out=zq_all[:, col:col + 1])
                            dm = dumpp.tile([128, QW], f32, tag="dm")
                            nc.vector.scalar_tensor_tensor(
                                out=dm[:],
                                in0=u_t[:, q * QW:(q + 1) * QW],
                                scalar=mtf[:],
                                in1=cp[:],
                                op0=Alu.is_le,
                                op1=Alu.mult,
                                accum_out=vq_all[:, col:col + 1])
                for col, mq in mqs:
                    nc.vector.tensor_copy(mq_all[:, col:col + 1], mq[:])

            NTQ = NT * NQ
            nc.sync.dma_start(out_d.ap()[:, 0:NTQ], vq_all[:])
            nc.sync.dma_start(out_d.ap()[:, NTQ:2 * NTQ], mq_all[:])
            nc.sync.dma_start(out_d.ap()[:, 2 * NTQ:3 * NTQ], zq_all[:])

    nc.compile()
    return nc


def _prep_host(pred_actions, target_actions, centers, mean, std,
               pca_components):
    f32 = np.float32
    mean = np.asarray(mean, f32)
    std = np.asarray(std, f32)
    pca = np.asarray(pca_components, f32)
    centers = np.asarray(centers, f32)
    inv_std = (1.0 / np.maximum(std, 1e-6)).astype(f32)
    w2 = (pca * (-2.0 * inv_std)[:, None]).astype(f32)
    w2h = w2.astype(F16)
    w2l = (w2 - w2h.astype(f32)).astype(F16)
    b0 = (-(mean * inv_std)) @ pca                      # [P]
    c2 = np.einsum("kp,kp->k", centers, centers)
    dneg = (c2 - 2.0 * (b0 @ centers.T)).astype(f32)    # [K]
    dneg = (dneg - np.float32(dneg.mean())).astype(f32)  # shift-invariant
    dh = dneg.astype(BF)
    dm = (dneg - dh.astype(f32)).astype(BF)
    dl = (dneg - dh.astype(f32) - dm.astype(f32)).astype(BF)
    dneg3 = np.ascontiguousarray(np.stack([dh, dm, dl], axis=0))  # [3, K]
    ones3 = np.ones((3, 128), dtype=BF)
    ctf = np.ascontiguousarray(centers.T).astype(f32)   # [P, K]
    cth = ctf.astype(F16)
    ctl = (ctf - cth.astype(f32)).astype(F16)

    def split(x):
        # hi/lo fp16 split, pre-transposed to [N_CORES, F, BS]
        x = np.asarray(x, f32).reshape(B, F)
        h = x.astype(F16)
        l = (x - h.astype(f32)).astype(F16)

        def shard_t(a):
            return np.ascontiguousarray(
                a.reshape(N_CORES, BS, F).transpose(0, 2, 1))

        return shard_t(h), shard_t(l)

    xth, xtl = split(target_actions)
    xph, xpl = split(pred_actions)
    return xth, xtl, xph, xpl, w2h, w2l, cth, ctl, dneg3, ones3


def run_device(xth, xtl, xph, xpl, w2h, w2l, cth, ctl, dneg3, ones3):
    from concourse.bass_utils import run_bass_kernel_spmd
    if "nc" not in _CACHE:
        _CACHE["nc"] = _build()
    nc = _CACHE["nc"]
    in_maps = []
    for c in range(N_CORES):
        in_maps.append({
            "xth": xth[c], "xtl": xtl[c], "xph": xph[c], "xpl": xpl[c],
            "w2h": w2h, "w2l": w2l, "cth": cth, "ctl": ctl,
            "dneg3": dneg3, "ones3": ones3,
        })
    res = run_bass_kernel_spmd(nc, in_maps, list(range(N_CORES)))
    return [r["res"] for r in res.results]


def reduce_host(outs):
    NTQ = NT * NQ
    loss_sum = 0.0
    acc_sum = 0
    for o in outs:
        v = o[:, 0:NTQ].reshape(128, NT, NQ)
        mq = o[:, NTQ:2 * NTQ].reshape(128, NT, NQ)
        zq = o[:, 2 * NTQ:3 * NTQ].reshape(128, NT, NQ)
        vsum = v.sum(axis=2)                       # u_neg_p at target argmin
        mp = mq.min(axis=2)                        # final m_neg_p
        # log Z with global shift: sum_q Zq * exp(m_neg_p - m_neg_q)
        z = (zq.astype(np.float64) * np.exp(
            (mp[:, :, None] - mq).astype(np.float64))).sum(axis=2)
        loss_rows = np.log(z) + (vsum - mp).astype(np.float64)
        loss_sum += loss_rows.sum()
        acc_sum += int((vsum == mp).sum())
    loss = np.float32(loss_sum / B)
    acc = np.float32(acc_sum / B)
    return loss, acc


def kernel(pred_actions, target_actions, centers, mean, std, pca_components):
    prepped = _prep_host(pred_actions, target_actions, centers, mean, std,
                         pca_components)
    outs = run_device(*prepped)
    return reduce_host(outs)
